# revision 24
# baseline (speedup 1.0000x reference)
"""Trainium2 Bass kernel for nn_Encoder_61830349193463 (retrieval_knn).

v2 strategy (data-parallel over src rows, 8 NeuronCores):
  - kNN scores via ONE bf16 matmul term (sh.ah) instead of the 3-term
    split-bf16 decomposition: 3x less PE work. The bf16 ranking error
    (~0.06 abs on sims with 5/6-gap scale ~4.5) is too big for a direct
    top-5, so we shortlist the top-8 candidates per row (DVE max8 over
    m-octants + merge), gather the 8 candidate anchor rows (fp32) and
    RESCORE them exactly: sim = s.a - 0.5*||a||^2 computed as a 520-dim
    fp32 dot (the norm term rides along as an extra column). Top-5 of
    the exact 8 -> indices -> gather + mean. Net ranking error ~= fp32.
  - Rescore work is split across engines: multiply on GpSimd, reductions
    on DVE/GpSimd, so the octant-7 tail pipelines under the PE.
  - The dense chain front (linear_dim + fusion) is emitted per 512-row
    block inside the octant-7 loop so the PE overlaps it with the tail.
  - Dense chain feature-major as v1; BN stats AllReduced across 8 cores.
"""

import numpy as np

import concourse.bacc as bacc
import concourse.bass as bass
import concourse.mybir as mybir
import concourse.tile as tile
from concourse.bass import IndirectOffsetOnAxis
from concourse.bass_utils import run_bass_kernel_spmd
from concourse.masks import make_identity
import ml_dtypes

F32 = mybir.dt.float32
BF16 = mybir.dt.bfloat16
U32 = mybir.dt.uint32
U16 = mybir.dt.uint16
AF = mybir.ActivationFunctionType
OP = mybir.AluOpType
P = 128

# problem sizes (hardcoded per contract)
N_FULL, M, D, F = 16384, 8192, 512, 2048
N_CORES = 8
K = 5
EPS = 1e-5
DEXT = 520          # 512 dims + norm column + pad
NCAND = 8           # candidates rescored per row


def build_kernel(ns=N_FULL // N_CORES, m=M, d=D, f=F, n_cores=N_CORES,
                 mo=1024):
    """Build the SPMD Bass module. ns/m/d/f sizes are per-core."""
    DC = d // P          # contraction chunks of the d dim (4)
    FC = f // P          # chunks of the hidden dim (16)
    T = ns // P          # n-tiles per core (16)
    OCT = m // mo        # m-octants (8)
    NQ = 8 * OCT         # candidates per tile before merge (64)
    nbf = 512            # n-block for dense matmuls
    NB = ns // nbf       # dense n-blocks (4)
    TPB = nbf // P       # tiles per dense block (4)
    NTOT = float(ns * n_cores)

    nc = bacc.Bacc("TRN2", target_bir_lowering=False, debug=False,
                   num_devices=n_cores)

    def param(name, shape, dt=F32):
        return nc.declare_dram_parameter(name, list(shape), dt, isOutput=False)

    srcT_h = param("srcT_h", [d, ns], BF16)
    src_ext = param("src_ext", [ns, DEXT], F32)    # [src | -1 | 0pad]
    anchT_h = param("anchT_h", [d, m], BF16)
    anch_ext = param("anch_ext", [m, DEXT], F32)   # [a | 0.5||a||^2 | 0pad]
    negam2 = param("negam2", [1, m], BF16)         # -0.5*||a||^2 row
    wdim = param("wdim", [d, d], BF16)             # pre-scaled by 1/K
    wfus = param("wfus", [2 * d, d], BF16)
    we1 = param("we1", [d, f], BF16)
    we2 = param("we2", [f, d], BF16)
    wd = param("wd", [d, d], BF16)
    bdim = param("bdim", [P, DC])
    bfus = param("bfus", [P, DC])
    be1 = param("be1", [P, FC])
    be2 = param("be2", [P, DC])
    bd = param("bd", [P, DC])
    g1 = param("g1", [P, DC]); bt1 = param("bt1", [P, DC])
    g2 = param("g2", [P, DC]); bt2 = param("bt2", [P, DC])
    gd = param("gd", [P, DC]); btd = param("btd", [P, DC])
    out = nc.declare_dram_parameter("out", [ns, d], F32, isOutput=True)

    # internal DRAM for the three BN-stat AllReduces
    cc_in = [nc.dram_tensor(f"cc{i}_in", [P, 2 * DC], F32) for i in range(3)]
    cc_space = "Shared" if n_cores > 4 else "Local"
    cc_out = [nc.dram_tensor(f"cc{i}_out", [P, 2 * DC], F32,
                             addr_space=cc_space) for i in range(3)]
    groups = [list(range(n_cores))]

    with tile.TileContext(nc) as tc:
        with (
            tc.tile_pool(name="persist", bufs=1) as pp,
            tc.tile_pool(name="wpool", bufs=1) as wp,
        ):
            ident = pp.tile([P, P], F32, name="ident")
            make_identity(nc, ident[:])

            # ---- resident tiles ----
            sTh = []
            for c in range(DC):
                th = pp.tile([P, ns], BF16, tag=f"sTh{c}", name=f"sTh{c}")
                nc.sync.dma_start(out=th[:], in_=srcT_h[c * P:(c + 1) * P, :])
                sTh.append(th)
            ones = pp.tile([1, P], BF16, tag="ones", name="ones")
            nc.vector.memset(ones[:], 1.0)
            identb = pp.tile([P, P], BF16, tag="identb", name="identb")
            nc.vector.tensor_copy(identb[:], ident[:])

            neighT = [pp.tile([P, ns], BF16, tag=f"nT{c}", name=f"nT{c}")
                      for c in range(DC)]
            combraw = [pp.tile([P, ns], BF16, tag=f"craw{c}", name=f"craw{c}")
                       for c in range(DC)]

            vcand = [pp.tile([P, NQ], F32, tag=f"vc{t}", name=f"vc{t}")
                     for t in range(T)]
            icand = [pp.tile([P, NQ], F32, tag=f"ic{t}", name=f"ic{t}")
                     for t in range(T)]

            # weights needed during the phase-A interleave
            def load_w(t_dram, rows, cols, tag):
                tiles = []
                for c in range(rows // P):
                    w = wp.tile([P, cols], BF16, tag=f"{tag}{c}",
                                name=f"{tag}{c}")
                    nc.scalar.dma_start(out=w[:],
                                        in_=t_dram[c * P:(c + 1) * P, :])
                    tiles.append(w)
                return tiles

            wdim_t = load_w(wdim, d, d, "wdim")
            wfus_t = load_w(wfus, 2 * d, d, "wfus")

            bias_t = {}
            for name, t_dram, cols in [
                    ("bdim", bdim, DC), ("bfus", bfus, DC), ("be1", be1, FC),
                    ("be2", be2, DC), ("bd", bd, DC), ("g1", g1, DC),
                    ("bt1", bt1, DC), ("g2", g2, DC), ("bt2", bt2, DC),
                    ("gd", gd, DC), ("btd", btd, DC)]:
                bt_ = wp.tile([P, cols], F32, tag=name, name=name)
                nc.scalar.dma_start(out=bt_[:], in_=t_dram[:, :])
                bias_t[name] = bt_

            # per-block BN stat partials: stats are linear over blocks, so
            # each block's sums are taken as soon as it is produced and only
            # the last block's stats serialize before the AllReduce
            stq = [[pp.tile([P, 2 * DC], F32, tag=f"stq{i}_{nb}",
                            name=f"stq{i}_{nb}") for nb in range(NB)]
                   for i in range(3)]
            sqscr = [pp.tile([P, nbf], BF16, tag="sqs0", name="sqs0")]

            def bn_partial(tiles, idx, nb):
                n_sl = slice(nb * nbf, (nb + 1) * nbf)
                st = stq[idx][nb]
                for c in range(DC):
                    nc.vector.tensor_reduce(out=st[:, c:c + 1],
                                            in_=tiles[c][:, n_sl],
                                            axis=mybir.AxisListType.X,
                                            op=OP.add)
                    nc.scalar.activation(sqscr[0][:], tiles[c][:, n_sl],
                                         AF.Square,
                                         accum_out=st[:, DC + c:DC + c + 1])

            # ================= PHASE A: kNN + rescore + dense front ========
            with (
                tc.tile_pool(name="aq", bufs=2) as aq_pool,
                tc.tile_pool(name="nam", bufs=2) as nam_pool,
                tc.tile_pool(name="dps", bufs=3, space="PSUM") as dps,
                tc.tile_pool(name="tops", bufs=4) as tops,
                tc.tile_pool(name="mrg", bufs=2) as mrg,
                tc.tile_pool(name="gat", bufs=2) as gat,
                tc.tile_pool(name="scr", bufs=1) as scr,
                tc.tile_pool(name="g5", bufs=2) as g5p,
                tc.tile_pool(name="sxt", bufs=1) as sxt,
                tc.tile_pool(name="amp", bufs=1) as amp,
                tc.tile_pool(name="tps", bufs=1, space="PSUM") as tpsp,
                tc.tile_pool(name="psA", bufs=1, space="PSUM") as psA,
            ):
                def emit_tail(t):
                    """merge -> gather8 -> exact rescore -> top5 -> mean."""
                    # global top-8 of the 64 candidate values
                    g8 = mrg.tile([P, 8], F32, tag="g8", name="g8")
                    nc.vector.max(out=g8[:], in_=vcand[t][:])
                    # match values back to candidate slots -> global indices
                    eqm = mrg.tile([P, 8 * NQ], F32, tag="eqm", name="eqm")
                    nc.vector.tensor_tensor(
                        out=eqm[:].rearrange("p (a b) -> p a b", a=8),
                        in0=vcand[t][:].rearrange(
                            "p (a q) -> p a q", a=1).to_broadcast([P, 8, NQ]),
                        in1=g8[:].rearrange(
                            "p (a o) -> p a o", o=1).to_broadcast([P, 8, NQ]),
                        op=OP.is_equal)
                    prod = mrg.tile([P, 8 * NQ], F32, tag="prod", name="prod")
                    nc.vector.tensor_tensor(
                        out=prod[:].rearrange("p (a b) -> p a b", a=8),
                        in0=eqm[:].rearrange("p (a b) -> p a b", a=8),
                        in1=icand[t][:].rearrange(
                            "p (a q) -> p a q", a=1).to_broadcast([P, 8, NQ]),
                        op=OP.mult)
                    idx8f = mrg.tile([P, 8], F32, tag="idx8f", name="idx8f")
                    nc.vector.tensor_reduce(
                        out=idx8f[:],
                        in_=prod[:].rearrange("p (a b) -> p a b", a=8),
                        axis=mybir.AxisListType.X, op=OP.add)
                    idx8 = mrg.tile([P, 8], U32, tag="idx8", name="idx8")
                    nc.vector.tensor_copy(idx8[:], idx8f[:])

                    # gather the 8 candidate rows [a | 0.5||a||^2 | pad]
                    G = gat.tile([P, NCAND * DEXT], F32, tag="G", name="G")
                    for k in range(NCAND):
                        nc.gpsimd.indirect_dma_start(
                            out=G[:, k * DEXT:(k + 1) * DEXT], out_offset=None,
                            in_=anch_ext[:],
                            in_offset=IndirectOffsetOnAxis(
                                ap=idx8[:, k:k + 1], axis=0))
                    sx = sxt.tile([P, DEXT], F32, tag="sx", name="sx")
                    nc.sync.dma_start(out=sx[:],
                                      in_=src_ext[t * P:(t + 1) * P, :])
                    # exact rescore: sims8[k] = sum_e G[k,e] * sx[e]
                    # fused multiply+sum on DVE (affine_mul_reduce)
                    sims8 = mrg.tile([P, 8], F32, tag="sims8", name="sims8")
                    sdot = scr.tile([P, DEXT], F32, tag="sdot", name="sdot")
                    for k in range(NCAND):
                        nc.vector.affine_mul_reduce(
                            out=sdot[:], accum_out=sims8[:, k:k + 1],
                            in0=G[:, k * DEXT:(k + 1) * DEXT], in1=sx[:],
                            scale=1.0, bias=0.0)
                    # exact top-5: mask = (sims8 >= 5th value), then the
                    # mean rides free on PSUM accumulation of the PE
                    # transposes of the ScalarE-masked candidates
                    v8r = mrg.tile([P, 8], F32, tag="v8r", name="v8r")
                    nc.vector.max(out=v8r[:], in_=sims8[:])
                    mask = mrg.tile([P, 8], F32, tag="mask", name="mask")
                    nc.vector.tensor_tensor(
                        out=mask[:],
                        in0=sims8[:],
                        in1=v8r[:, 4:5].to_broadcast([P, 8]),
                        op=OP.is_ge)
                    Gm = g5p.tile([P, NCAND * d], F32, tag="Gm", name="Gm")
                    for k in range(NCAND):
                        nc.scalar.activation(
                            Gm[:, k * d:(k + 1) * d],
                            G[:, k * DEXT:k * DEXT + d],
                            AF.Identity, scale=mask[:, k:k + 1])
                    tps = tpsp.tile([P, d], F32, name="tps")
                    for j in range(DC):
                        for k in range(NCAND):
                            nc.tensor.matmul(
                                tps[:, j * P:(j + 1) * P],
                                Gm[:, k * d + j * P:k * d + (j + 1) * P],
                                ident[:], start=(k == 0),
                                stop=(k == NCAND - 1), is_transpose=True)
                    for j in range(DC):
                        nc.scalar.copy(neighT[j][:, t * P:(t + 1) * P],
                                       tps[:, j * P:(j + 1) * P])

                def emit_dense_front(nb):
                    """linear_dim + fusion for n-block nb (overlaps tail)."""
                    n_sl = slice(nb * nbf, (nb + 1) * nbf)
                    amapT = [amp.tile([P, nbf], BF16, tag=f"amap{c}",
                                      name=f"amap{c}") for c in range(DC)]
                    for fc in range(DC):
                        ps = psA.tile([P, nbf], F32, tag="psF", name="psF")
                        for c in range(DC):
                            nc.tensor.matmul(
                                ps[:], wdim_t[c][:, fc * P:(fc + 1) * P],
                                neighT[c][:, n_sl],
                                start=(c == 0), stop=(c == DC - 1))
                        nc.scalar.activation(amapT[fc][:], ps[:],
                                             AF.Identity,
                                             bias=bias_t["bdim"][:, fc:fc + 1])
                    for fc in range(DC):
                        ps = psA.tile([P, nbf], F32, tag="psF", name="psF")
                        for c in range(2 * DC):
                            rhs = sTh[c][:, n_sl] if c < DC else \
                                amapT[c - DC][:]
                            nc.tensor.matmul(
                                ps[:], wfus_t[c][:, fc * P:(fc + 1) * P], rhs,
                                start=(c == 0), stop=(c == 2 * DC - 1))
                        nc.scalar.activation(combraw[fc][:, n_sl], ps[:],
                                             AF.Identity,
                                             bias=bias_t["bfus"][:, fc:fc + 1])
                    bn_partial(combraw, 0, nb)

                for o in range(OCT):
                    aqh = [aq_pool.tile([P, mo], BF16, tag=f"aqh{c}",
                                        name=f"aqh{c}") for c in range(DC)]
                    for c in range(DC):
                        nc.sync.dma_start(
                            out=aqh[c][:],
                            in_=anchT_h[c * P:(c + 1) * P, o * mo:(o + 1) * mo])
                    nam = nam_pool.tile([1, mo], BF16, tag="nam", name="nam")
                    nc.sync.dma_start(out=nam[:],
                                      in_=negam2[:, o * mo:(o + 1) * mo])
                    for t in range(T):
                        # sims accumulate in PSUM: -0.5||a||^2 via a K=1
                        # ones-row matmul, then the 4 contraction chunks
                        ps = dps.tile([P, mo], F32, name="dps")
                        n_sl = slice(t * P, (t + 1) * P)
                        for h in range(mo // 512):
                            h_sl = slice(h * 512, (h + 1) * 512)
                            nc.tensor.matmul(ps[:, h_sl], ones[:],
                                             nam[:, h_sl],
                                             start=True, stop=False)
                            for c in range(DC):
                                nc.tensor.matmul(ps[:, h_sl],
                                                 sTh[c][:, n_sl],
                                                 aqh[c][:, h_sl],
                                                 start=False,
                                                 stop=(c == DC - 1))
                        # top-8 of this octant read directly from PSUM
                        nc.vector.max(out=vcand[t][:, o * 8:(o + 1) * 8],
                                      in_=ps[:])
                        i16 = tops.tile([P, 8], U16, tag="i16", name="i16")
                        nc.vector.max_index(
                            out=i16[:], in_max=vcand[t][:, o * 8:(o + 1) * 8],
                            in_values=ps[:])
                        nc.vector.tensor_scalar(
                            out=icand[t][:, o * 8:(o + 1) * 8], in0=i16[:],
                            scalar1=float(o * mo), scalar2=None, op0=OP.add)
                        if o == OCT - 1:
                            emit_tail(t)
                            if t % TPB == TPB - 1:
                                emit_dense_front(t // TPB)

            # ================= PHASE B: rest of the dense chain ============
            we1_t = load_w(we1, d, f, "we1")
            we2_t = load_w(we2, f, d, "we2")
            wd_t = load_w(wd, d, d, "wd")

            with (
                tc.tile_pool(name="act", bufs=1) as ap_,
                tc.tile_pool(name="mlp", bufs=1) as mp_,
                tc.tile_pool(name="bps", bufs=4, space="PSUM") as bps,
                tc.tile_pool(name="stat", bufs=1) as stp,
            ):
                def bn_finish(idx):
                    st = stp.tile([P, 2 * DC], F32, tag=f"st{idx}",
                                  name=f"st{idx}")
                    nc.vector.tensor_tensor(out=st[:], in0=stq[idx][0][:],
                                            in1=stq[idx][1][:], op=OP.add)
                    for nb in range(2, NB):
                        nc.vector.tensor_tensor(out=st[:], in0=st[:],
                                                in1=stq[idx][nb][:], op=OP.add)
                    nc.sync.dma_start(out=cc_in[idx][:], in_=st[:])
                    nc.gpsimd.collective_compute(
                        "AllReduce", OP.add, replica_groups=groups,
                        ins=[cc_in[idx].ap()], outs=[cc_out[idx].ap()])
                    gst = stp.tile([P, 2 * DC], F32, tag=f"gst{idx}",
                                   name=f"gst{idx}")
                    nc.sync.dma_start(out=gst[:], in_=cc_out[idx][:])
                    mu = stp.tile([P, DC], F32, tag=f"mu{idx}", name=f"mu{idx}")
                    nc.vector.tensor_scalar(out=mu[:], in0=gst[:, :DC],
                                            scalar1=1.0 / NTOT, scalar2=None,
                                            op0=OP.mult)
                    musq = stp.tile([P, DC], F32, tag=f"musq{idx}",
                                    name=f"musq{idx}")
                    nc.vector.tensor_tensor(out=musq[:], in0=mu[:], in1=mu[:],
                                            op=OP.mult)
                    var = stp.tile([P, DC], F32, tag=f"var{idx}",
                                   name=f"var{idx}")
                    nc.vector.scalar_tensor_tensor(
                        out=var[:], in0=gst[:, DC:], scalar=1.0 / NTOT,
                        in1=musq[:], op0=OP.mult, op1=OP.subtract)
                    sd = stp.tile([P, DC], F32, tag=f"sd{idx}", name=f"sd{idx}")
                    nc.vector.tensor_scalar(out=sd[:], in0=var[:], scalar1=EPS,
                                            scalar2=None, op0=OP.add)
                    nc.scalar.sqrt(sd[:], sd[:])
                    rs = stp.tile([P, DC], F32, tag=f"rs{idx}", name=f"rs{idx}")
                    nc.vector.reciprocal(rs[:], sd[:])
                    return mu, rs

                def bn_affine(mu, rs, gname, bname, idx):
                    s = stp.tile([P, DC], F32, tag=f"s{idx}", name=f"s{idx}")
                    nc.vector.tensor_tensor(out=s[:], in0=rs[:],
                                            in1=bias_t[gname][:], op=OP.mult)
                    tmp = stp.tile([P, DC], F32, tag=f"tmp{idx}",
                                   name=f"tmp{idx}")
                    nc.vector.tensor_tensor(out=tmp[:], in0=mu[:], in1=s[:],
                                            op=OP.mult)
                    tb = stp.tile([P, DC], F32, tag=f"tb{idx}", name=f"tb{idx}")
                    nc.vector.tensor_tensor(out=tb[:], in0=bias_t[bname][:],
                                            in1=tmp[:], op=OP.subtract)
                    return s, tb

                mu1, rs1 = bn_finish(0)
                s1, t1 = bn_affine(mu1, rs1, "g1", "bt1", 0)
                combT = [ap_.tile([P, ns], BF16, tag=f"combT{c}",
                                  name=f"combT{c}") for c in range(DC)]

                r2T = [ap_.tile([P, ns], BF16, tag=f"r2T{c}", name=f"r2T{c}")
                       for c in range(DC)]
                for nb in range(NB):
                    n_sl = slice(nb * nbf, (nb + 1) * nbf)
                    for c in range(DC):
                        nc.scalar.activation(combT[c][:, n_sl],
                                             combraw[c][:, n_sl],
                                             AF.Identity, bias=t1[:, c:c + 1],
                                             scale=s1[:, c:c + 1])
                    tT = [mp_.tile([P, nbf], BF16, tag=f"tT{fe}",
                                   name=f"tT{fe}") for fe in range(FC)]
                    for fe in range(FC):
                        ps = bps.tile([P, nbf], F32, tag="psB", name="psB")
                        for c in range(DC):
                            nc.tensor.matmul(
                                ps[:], we1_t[c][:, fe * P:(fe + 1) * P],
                                combT[c][:, n_sl],
                                start=(c == 0), stop=(c == DC - 1))
                        nc.scalar.activation(tT[fe][:], ps[:], AF.Tanh,
                                             bias=bias_t["be1"][:, fe:fe + 1])
                    for fc in range(DC):
                        ps = bps.tile([P, nbf], F32, tag="psB", name="psB")
                        for fe in range(FC):
                            nc.tensor.matmul(
                                ps[:], we2_t[fe][:, fc * P:(fc + 1) * P],
                                tT[fe][:],
                                start=(fe == 0), stop=(fe == FC - 1))
                        nc.vector.scalar_tensor_tensor(
                            out=r2T[fc][:, n_sl], in0=ps[:],
                            scalar=bias_t["be2"][:, fc:fc + 1],
                            in1=combT[fc][:, n_sl], op0=OP.add, op1=OP.add)
                    bn_partial(r2T, 1, nb)

                mu2, rs2 = bn_finish(1)
                s2, t2 = bn_affine(mu2, rs2, "g2", "bt2", 1)
                c2T = combraw  # reuse buffers

                yT = [ap_.tile([P, ns], BF16, tag=f"yT{c}", name=f"yT{c}")
                      for c in range(DC)]
                for nb in range(NB):
                    n_sl = slice(nb * nbf, (nb + 1) * nbf)
                    for c in range(DC):
                        nc.scalar.activation(c2T[c][:, n_sl],
                                             r2T[c][:, n_sl], AF.Identity,
                                             bias=t2[:, c:c + 1],
                                             scale=s2[:, c:c + 1])
                    for fc in range(DC):
                        ps = bps.tile([P, nbf], F32, tag="psB", name="psB")
                        for c in range(DC):
                            nc.tensor.matmul(
                                ps[:], wd_t[c][:, fc * P:(fc + 1) * P],
                                c2T[c][:, n_sl],
                                start=(c == 0), stop=(c == DC - 1))
                        nc.scalar.activation(yT[fc][:, n_sl], ps[:],
                                             AF.Identity,
                                             bias=bias_t["bd"][:, fc:fc + 1])
                    bn_partial(yT, 2, nb)

                mu3, rs3 = bn_finish(2)
                s3, t3 = bn_affine(mu3, rs3, "gd", "btd", 2)

                with (
                    tc.tile_pool(name="ops", bufs=2, space="PSUM") as opsp,
                    tc.tile_pool(name="onat", bufs=3) as onp,
                ):
                    for t in range(T):
                        otmp = onp.tile([P, d], F32, tag="otmp", name="otmp")
                        for j in range(DC):
                            nc.scalar.activation(
                                otmp[:, j * P:(j + 1) * P],
                                yT[j][:, t * P:(t + 1) * P], AF.Tanh,
                                bias=t3[:, j:j + 1], scale=s3[:, j:j + 1])
                        tps = opsp.tile([P, d], F32, name="otps")
                        for j in range(DC):
                            nc.tensor.transpose(
                                out=tps[:, j * P:(j + 1) * P],
                                in_=otmp[:, j * P:(j + 1) * P],
                                identity=ident[:])
                        onat = onp.tile([P, d], F32, tag="onat", name="onat")
                        nc.scalar.copy(onat[:], tps[:])
                        nc.sync.dma_start(out=out[t * P:(t + 1) * P, :],
                                          in_=onat[:])

    nc.finalize()
    return nc


def _chunk_vec(v, cols):
    # [cols*128] feature vector -> [128, cols] feature-major chunk layout
    return np.ascontiguousarray(v.reshape(cols, P).T)


def prepare_inputs(src, anchor_2, W_dim, b_dim, W_fus, b_fus, W_e1, b_e1,
                   W_e2, b_e2, g1, bt1, g2, bt2, W_d, b_d, g_d, bt_d,
                   n_cores=N_CORES, ns=N_FULL // N_CORES):
    """Host-side prep: shard + transpose + bf16 rounding + layout."""
    d = src.shape[1]
    f = W_e1.shape[1]
    m = anchor_2.shape[0]
    DC, FC = d // P, f // P
    am2 = 0.5 * (anchor_2.astype(np.float64) ** 2).sum(1).astype(np.float32)
    negam2 = (-am2[None, :]).astype(ml_dtypes.bfloat16)
    anch_ext = np.zeros((m, DEXT), np.float32)
    anch_ext[:, :d] = anchor_2
    anch_ext[:, d] = am2
    shared = dict(
        anchT_h=anchor_2.T.astype(ml_dtypes.bfloat16),
        anch_ext=anch_ext,
        negam2=negam2,
        wdim=(W_dim / K).astype(ml_dtypes.bfloat16),
        wfus=W_fus.astype(ml_dtypes.bfloat16),
        we1=W_e1.astype(ml_dtypes.bfloat16),
        we2=W_e2.astype(ml_dtypes.bfloat16),
        wd=W_d.astype(ml_dtypes.bfloat16),
        bdim=_chunk_vec(b_dim, DC), bfus=_chunk_vec(b_fus, DC),
        be1=_chunk_vec(b_e1, FC), be2=_chunk_vec(b_e2, DC),
        bd=_chunk_vec(b_d, DC),
        g1=_chunk_vec(g1, DC), bt1=_chunk_vec(bt1, DC),
        g2=_chunk_vec(g2, DC), bt2=_chunk_vec(bt2, DC),
        gd=_chunk_vec(g_d, DC), btd=_chunk_vec(bt_d, DC),
    )
    in_maps = []
    for c in range(n_cores):
        shard = src[c * ns:(c + 1) * ns]
        sx = np.zeros((ns, DEXT), np.float32)
        sx[:, :d] = shard
        sx[:, d] = -1.0
        in_maps.append(dict(
            shared,
            srcT_h=np.ascontiguousarray(shard.T).astype(ml_dtypes.bfloat16),
            src_ext=sx))
    return in_maps


_NC_CACHE = {}


def kernel(**inputs):
    key = "full"
    if key not in _NC_CACHE:
        _NC_CACHE[key] = build_kernel()
    nc = _NC_CACHE[key]
    in_maps = prepare_inputs(**{k: np.asarray(v) for k, v in inputs.items()})
    res = run_bass_kernel_spmd(nc, in_maps, core_ids=list(range(N_CORES)))
    return np.concatenate([r["out"] for r in res.results], axis=0)
